# revision 2
# baseline (speedup 1.0000x reference)
"""Trainium2 Bass kernel for nn_CustomProposalLayer (YOLOv4-style decode + per-image greedy NMS).

Optimized v2. Data-parallel over batch: 4 images per core on 8 cores.

Pipeline per core:
  A. Host pre-lays conf/cls planes in the exact [128, F] topk layout; the
     kernel streams them with large contiguous DMAs, computing
     sigmoid(conf)*sigmoid(cls) on ACT+DVE into S.
  B. One GPSIMD topk -> best-128 slot ids per half-image; the index half is
     relaid out through one small DRAM bounce (2 DMAs).
  C. 8 per-column indirect gathers fetch the 1024 candidates' raw 6-field
     predictions. Grid/anchor/stride constants are recomputed on DVE straight
     from the slot id (no table gather).
  D. Polynomial Cody-Waite exp gives f32 exp(tw/th) (~1e-9 rel) and a
     double-float e^-conf / e^-cls; rank keys D = (1+e^-conf)(1+e^-cls) in
     double-float order candidates exactly like the reference f32 scores
     (ascending D = descending score), with the flat index as tie-break.
  E. Per image: PE transpose+broadcast replicate the rank keys and sorted
     boxes across partitions (no DRAM bounces), DVE computes the 256-way
     rank and the 128x128 IoU matrix, a fixed-point PE iteration reproduces
     greedy-NMS keep flags, and one-hot matmuls compact the output.
"""

import functools
import os
from contextlib import ExitStack

import numpy as np

DEBUG_DUMP = bool(int(os.environ.get("KV2_DEBUG", "0")))

import concourse.bass as bass
import concourse.bacc as bacc
import concourse.mybir as mybir
from concourse import tile
from concourse.ap import AP
from concourse.bass_utils import run_bass_kernel_spmd
from concourse import library_config

f32 = mybir.dt.float32
u32 = mybir.dt.uint32

# ---- problem geometry (hardcoded; spec.json shapes) ----
B, CORES, IPC = 32, 8, 4          # batch, cores, images per core
A = 4
LV_W = (152, 76, 38, 19)
N = sum(A * w * w for w in LV_W)                # 122740
LV_BASE = (0, 92416, 115520, 121296)
STRIDES = (4.0, 8.0, 16.0, 32.0)
ANCHORS = np.array([
    [[12, 16], [19, 36], [40, 28], [36, 75]],
    [[36, 75], [76, 55], [72, 146], [142, 110]],
    [[72, 146], [142, 110], [192, 243], [459, 401]],
    [[142, 110], [192, 243], [300, 300], [459, 401]],
], dtype=np.float32)
F = 3976                                        # score cols per partition
VOCAB = 16 * F                                  # 63616 per token (half-image)
K = 256
MAXP = 100
NMS_ITERS = 4                                   # fixed-point iterations (measured max 3)
FILL = -30.0                                    # filler logit -> score ~1e-26

# Cody-Waite exp constants
LOG2E = 1.4426950408889634
MAGIC = 12582912.0                              # 1.5 * 2^23
LN2HI = 0.693145751953125                       # 19-bit ln2 (exact * |k| < 32)
LN2R = -1.4286068202862268e-06                  # ln2hi - ln2 (r_lo = kf*LN2R)
QC = (1.0 / 40320, 1.0 / 5040, 1.0 / 720, 1.0 / 120, 1.0 / 24, 1.0 / 6, 0.5)


@functools.cache
def _tables():
    iota_row = np.tile(np.arange(128, dtype=np.float32), (128, 1))
    ltri = (np.arange(128)[:, None] <= np.arange(128)[None, :]).astype(np.float32)
    ident = np.eye(128, dtype=np.float32)
    imgb = np.zeros((128, 8), np.uint32)
    hoff = np.zeros((128, 8), np.uint32)
    for b_ in range(8):
        imgb[:, b_] = (b_ // 2) * N
        hoff[:, b_] = (b_ % 2) * VOCAB
    # jmat gate masks: maskJ[i][q, 256f+128h+j] = (q == 8f + 2i + h)
    maskJ = np.zeros((IPC, 24, 512), np.float32)
    for i in range(IPC):
        for f_ in range(2):
            for h_ in range(2):
                maskJ[i, 8 * f_ + 2 * i + h_, 256 * f_ + 128 * h_:256 * f_ + 128 * h_ + 128] = 1.0
    # jb gate mask: maskB[q, 128k+j] = (q == field_k), fields (0,1,2,3,5)
    maskB = np.zeros((8, 640), np.float32)
    for k_, f_ in enumerate((0, 1, 2, 3, 5)):
        maskB[f_, 128 * k_:128 * (k_ + 1)] = 1.0
    ones24 = np.ones((24, 128), np.float32)
    return iota_row, ltri, ident, imgb, hoff, maskJ, maskB, ones24


# ------------------------------------------------------------- program build
def _body(nc: bass.Bass, tc: "tile.TileContext", es: ExitStack, x, s, out, st):
    iota_np, ltri_np, ident_np, imgb_np, hoff_np, maskJ_np, maskB_np, ones24_np = _tables()
    iota_h = nc.inline_tensor(iota_np, "c_iota")
    ltri_h = nc.inline_tensor(ltri_np, "c_ltri")
    ident_h = nc.inline_tensor(ident_np, "c_ident")
    imgb_h = nc.inline_tensor(imgb_np, "c_imgb")
    hoff_h = nc.inline_tensor(hoff_np, "c_hoff")
    maskJ_h = nc.inline_tensor(
        np.ascontiguousarray(np.transpose(maskJ_np, (1, 0, 2))).reshape(24, IPC * 512),
        "c_maskJ",
    )
    maskB_h = nc.inline_tensor(maskB_np, "c_maskB")
    ones24_h = nc.inline_tensor(ones24_np, "c_ones24")

    x_ap = x.ap()                                # [IPC*N*6] f32
    xg = x_ap.rearrange("(r f) -> r f", f=6)     # [IPC*N, 6] gather view
    out_ap = out.ap()                            # [IPC*MAXP*5] f32


    OP = mybir.AluOpType
    SIG = mybir.ActivationFunctionType.Sigmoid
    TT = nc.vector.tensor_tensor
    TS = nc.vector.tensor_scalar
    STT = nc.vector.scalar_tensor_tensor
    CP = nc.vector.tensor_copy

    # ---------------- stage A: scores into topk layout ----------------
    # input DMAs are issued before the constant loads so the score planes
    # hit the DMA queues first
    S_h = nc.alloc_sbuf_tensor("S_sb", [128, F], f32)
    S = S_h.ap()
    apool = es.enter_context(tc.tile_pool(name="apool", bufs=2))
    NCH = 2
    CW = F // NCH                                # 1988
    uts, vts = [], []
    for k in range(NCH):
        u = apool.tile([128, CW], f32, tag="u", name=f"u_{k}")
        v = apool.tile([128, CW], f32, tag="v", name=f"v_{k}")
        nc.sync.dma_start(out=u[:], in_=AP(s, k * CW, [[F, 128], [1, CW]]))
        nc.sync.dma_start(out=v[:], in_=AP(s, 128 * F + k * CW, [[F, 128], [1, CW]]))
        uts.append(u); vts.append(v)

    cpool = es.enter_context(tc.tile_pool(name="consts", bufs=1))
    iota_sb = cpool.tile([128, 128], f32, name="iota_sb")
    ltri_sb = cpool.tile([128, 128], f32, name="ltri_sb")
    ident_sb = cpool.tile([128, 128], f32, name="ident_sb")
    maskJ_sb = cpool.tile([24, IPC * 512], f32, name="maskJ_sb")
    maskB_sb = cpool.tile([8, 640], f32, name="maskB_sb")
    ones24_sb = cpool.tile([24, 128], f32, name="ones24_sb")
    imgb_sb = cpool.tile([128, 8], u32, name="imgb_sb")
    hoff_sb = cpool.tile([128, 8], u32, name="hoff_sb")
    nc.sync.dma_start(out=iota_sb[:], in_=iota_h.ap())
    nc.sync.dma_start(out=ltri_sb[:], in_=ltri_h.ap())
    nc.sync.dma_start(out=ident_sb[:], in_=ident_h.ap())
    nc.sync.dma_start(out=maskJ_sb[:], in_=maskJ_h.ap())
    nc.sync.dma_start(out=maskB_sb[:], in_=maskB_h.ap())
    nc.sync.dma_start(out=ones24_sb[:], in_=ones24_h.ap())
    nc.sync.dma_start(out=imgb_sb[:], in_=imgb_h.ap())
    nc.sync.dma_start(out=hoff_sb[:], in_=hoff_h.ap())

    for k in range(NCH):
        u, v = uts[k], vts[k]
        su = apool.tile([128, CW], f32, tag="su", name=f"su_{k}")
        sv = apool.tile([128, CW], f32, tag="sv", name=f"sv_{k}")
        nc.scalar.activation(out=su[:], in_=u[:], func=SIG)
        nc.scalar.activation(out=sv[:], in_=v[:], func=SIG)
        TT(out=S[:, k * CW : (k + 1) * CW], in0=su[:], in1=sv[:], op=OP.mult)

    # ---------------- stage B: topk + index relayout ----------------
    gpool = es.enter_context(tc.tile_pool(name="gpool", bufs=1))
    tk_h = nc.alloc_sbuf_tensor("tk_sb", [128, 32], u32)
    tk = tk_h.ap()
    nc.gpsimd.topk(out_ap=tk, in_ap=S, tokens=8, vocab_size=VOCAB, k=K)

    # bounce idx half: st[q*16 + c] = tk[q, 16+c]; then
    # slotidx[p, b] = tk[16b+8+p//16, 16+p%16] = st[256b + 128 + p]
    nc.sync.dma_start(
        out=st.ap()[0:2048].rearrange("(q c) -> q c", c=16), in_=tk[:, 16:32]
    )
    slotidx = gpool.tile([128, 8], u32, name="slotidx")
    nc.sync.dma_start(out=slotidx[:], in_=AP(st, 128, [[1, 128], [256, 8]]))
    # cidx = h*VOCAB + slot == flat in-image position g
    cidx = gpool.tile([128, 8], u32, name="cidx")
    TT(out=cidx[:], in0=slotidx[:], in1=hoff_sb[:], op=OP.add)

    # ---------------- stage C: raw gathers + per-slot constants ----------------
    dpool = es.enter_context(tc.tile_pool(name="dpool", bufs=1))

    def dt(name, w=8):
        return dpool.tile([128, w], f32, name=name)

    rawidx = gpool.tile([128, 8], u32, name="rawidx")
    TT(out=rawidx[:], in0=cidx[:], in1=imgb_sb[:], op=OP.add)
    TS(out=rawidx[:], in0=rawidx[:], scalar1=IPC * N - 1, scalar2=None, op0=OP.min)
    raw = gpool.tile([128, 48], f32, name="raw")
    for b_ in range(8):
        nc.gpsimd.indirect_dma_start(
            out=raw[:, 6 * b_ : 6 * b_ + 6], out_offset=None, in_=xg,
            in_offset=bass.IndirectOffsetOnAxis(ap=rawidx[:, b_ : b_ + 1], axis=0),
        )

    # per-slot constants from g (all DVE; overlaps the gathers above)
    gf = dt("gf")
    CP(out=gf[:], in_=cidx[:])
    m1, m2, m3 = dt("m1"), dt("m2"), dt("m3")
    TS(out=m1[:], in0=gf[:], scalar1=float(LV_BASE[1]), scalar2=None, op0=OP.is_ge)
    TS(out=m2[:], in0=gf[:], scalar1=float(LV_BASE[2]), scalar2=None, op0=OP.is_ge)
    TS(out=m3[:], in0=gf[:], scalar1=float(LV_BASE[3]), scalar2=None, op0=OP.is_ge)
    pos = dt("pos")
    STT(out=pos[:], in0=m1[:], scalar=-float(LV_BASE[1]), in1=gf[:], op0=OP.mult, op1=OP.add)
    STT(out=pos[:], in0=m2[:], scalar=-float(LV_BASE[2] - LV_BASE[1]), in1=pos[:], op0=OP.mult, op1=OP.add)
    STT(out=pos[:], in0=m3[:], scalar=-float(LV_BASE[3] - LV_BASE[2]), in1=pos[:], op0=OP.mult, op1=OP.add)
    wl = dt("wl")
    TS(out=wl[:], in0=m1[:], scalar1=-76.0, scalar2=152.0, op0=OP.mult, op1=OP.add)
    STT(out=wl[:], in0=m2[:], scalar=-38.0, in1=wl[:], op0=OP.mult, op1=OP.add)
    STT(out=wl[:], in0=m3[:], scalar=-19.0, in1=wl[:], op0=OP.mult, op1=OP.add)
    w2 = dt("w2")
    TT(out=w2[:], in0=wl[:], in1=wl[:], op=OP.mult)
    # trunc(pos / w2) via level-masked reciprocals + bias (no HW tensor divide);
    # bias sits between the product rounding error and the minimum fraction 1/w2
    qt = dt("qt")
    qu = dpool.tile([128, 8], u32, name="qu")
    ai = dt("ai")
    r2c = dt("r2c")

    def level_masked(vals, out_t):
        """out = vals[lv] via cumulative level masks."""
        TS(out=out_t[:], in0=m1[:], scalar1=float(vals[1] - vals[0]),
           scalar2=float(vals[0]), op0=OP.mult, op1=OP.add)
        STT(out=out_t[:], in0=m2[:], scalar=float(vals[2] - vals[1]), in1=out_t[:],
            op0=OP.mult, op1=OP.add)
        STT(out=out_t[:], in0=m3[:], scalar=float(vals[3] - vals[2]), in1=out_t[:],
            op0=OP.mult, op1=OP.add)

    # HW f32->int convert rounds (CoreSim truncates) -- correct the quotient
    # with a branchless fixup so both semantics yield an exact floor.
    mfx = dt("mfx")
    sfx = dt("sfx")

    def floor_div(num, den, recip_vals, quot, remo):
        level_masked(recip_vals, r2c)
        TT(out=qt[:], in0=num[:], in1=r2c[:], op=OP.mult)
        TS(out=qt[:], in0=qt[:], scalar1=1e-5, scalar2=None, op0=OP.add)
        CP(out=qu[:], in_=qt[:])
        CP(out=quot[:], in_=qu[:])
        TT(out=remo[:], in0=quot[:], in1=den[:], op=OP.mult)
        TT(out=remo[:], in0=num[:], in1=remo[:], op=OP.subtract)
        TS(out=mfx[:], in0=remo[:], scalar1=0.0, scalar2=None, op0=OP.is_lt)
        TT(out=quot[:], in0=quot[:], in1=mfx[:], op=OP.subtract)
        TT(out=sfx[:], in0=mfx[:], in1=den[:], op=OP.mult)
        TT(out=remo[:], in0=remo[:], in1=sfx[:], op=OP.add)

    rem = dt("rem")
    floor_div(pos, w2, [1.0 / (w * w) for w in LV_W], ai, rem)
    gyf = dt("gyf")
    gxf = dt("gxf")
    floor_div(rem, wl, [1.0 / w for w in LV_W], gyf, gxf)
    stf = dt("stf")
    TS(out=stf[:], in0=m1[:], scalar1=4.0, scalar2=4.0, op0=OP.mult, op1=OP.add)
    STT(out=stf[:], in0=m2[:], scalar=8.0, in1=stf[:], op0=OP.mult, op1=OP.add)
    STT(out=stf[:], in0=m3[:], scalar=16.0, in1=stf[:], op0=OP.mult, op1=OP.add)
    # anchors: aw/ah = B + C1*n1 + C2*n2 + C3*n3 with level-masked coeffs
    n1, n2, n3 = dt("n1"), dt("n2"), dt("n3")
    TS(out=n1[:], in0=ai[:], scalar1=1.0, scalar2=None, op0=OP.is_ge)
    TS(out=n2[:], in0=ai[:], scalar1=2.0, scalar2=None, op0=OP.is_ge)
    TS(out=n3[:], in0=ai[:], scalar1=3.0, scalar2=None, op0=OP.is_ge)

    def anchor_field(col, out_t, scratch):
        tab = ANCHORS[:, :, col]  # [4 levels, 4 anchors]
        level_masked([tab[l][0] for l in range(4)], out_t)
        for a_, nm in ((1, n1), (2, n2), (3, n3)):
            level_masked([tab[l][a_] - tab[l][a_ - 1] for l in range(4)], scratch)
            TT(out=scratch[:], in0=scratch[:], in1=nm[:], op=OP.mult)
            TT(out=out_t[:], in0=out_t[:], in1=scratch[:], op=OP.add)

    awf, ahf, scr = dt("awf"), dt("ahf"), dt("scr")
    anchor_field(0, awf, scr)
    anchor_field(1, ahf, scr)

    # ---------------- stage D: exp / double-float keys ----------------
    raw3 = raw[:].rearrange("p (b f) -> p b f", f=6)
    a_fb = raw[:].rearrange("p (b f) -> p f b", f=6)

    def dt32(name):
        return dpool.tile([128, 32], f32, name=name)

    x4 = dt32("x4")
    CP(out=x4[:, 0:16], in_=a_fb[:, 2:4, :])
    nc.vector.tensor_scalar_mul(out=x4[:, 16:32], in0=a_fb[:, 4:6, :], scalar1=-1.0)
    kf, rh, rl, q_, t_ = dt32("kf"), dt32("rh"), dt32("rl"), dt32("q_"), dt32("t_")
    TS(out=t_[:], in0=x4[:], scalar1=LOG2E, scalar2=MAGIC, op0=OP.mult, op1=OP.add)
    TS(out=kf[:], in0=t_[:], scalar1=MAGIC, scalar2=None, op0=OP.subtract)
    TS(out=rh[:], in0=kf[:], scalar1=LN2HI, scalar2=None, op0=OP.mult)
    TT(out=rh[:], in0=x4[:], in1=rh[:], op=OP.subtract)
    TS(out=rl[:], in0=kf[:], scalar1=LN2R, scalar2=None, op0=OP.mult)
    TS(out=q_[:], in0=rh[:], scalar1=QC[0], scalar2=QC[1], op0=OP.mult, op1=OP.add)
    for c_ in QC[2:]:
        TT(out=q_[:], in0=q_[:], in1=rh[:], op=OP.mult)
        TS(out=q_[:], in0=q_[:], scalar1=float(c_), scalar2=None, op0=OP.add)
    r2, corr = dt32("r2"), dt32("corr")
    TT(out=r2[:], in0=rh[:], in1=rh[:], op=OP.mult)
    TT(out=corr[:], in0=r2[:], in1=q_[:], op=OP.mult)
    s_, e1, el = dt32("s_"), dt32("e1"), dt32("el")
    TS(out=s_[:], in0=rh[:], scalar1=1.0, scalar2=None, op0=OP.add)
    TS(out=e1[:], in0=s_[:], scalar1=1.0, scalar2=None, op0=OP.subtract)
    TT(out=e1[:], in0=rh[:], in1=e1[:], op=OP.subtract)
    TT(out=el[:], in0=e1[:], in1=corr[:], op=OP.add)
    TT(out=t_[:], in0=s_[:], in1=el[:], op=OP.add)
    TT(out=t_[:], in0=rl[:], in1=t_[:], op=OP.mult)
    TT(out=el[:], in0=el[:], in1=t_[:], op=OP.add)
    pw = dt32("pw")
    pwu = dpool.tile([128, 32], u32, name="pwu")
    TS(out=pw[:], in0=kf[:], scalar1=127.0, scalar2=8388608.0, op0=OP.add, op1=OP.mult)
    CP(out=pwu[:], in_=pw[:])
    pwf = pwu[:].bitcast(f32)
    # f32 exp for tw/th
    ew = dpool.tile([128, 16], f32, name="ew")
    TT(out=ew[:], in0=s_[:, 0:16], in1=el[:, 0:16], op=OP.add)
    TT(out=ew[:], in0=ew[:], in1=pwf[:, 0:16], op=OP.mult)
    # double-float E = e^-conf, e^-cls (exact pow2 scale)
    Eh = dpool.tile([128, 16], f32, name="Eh")
    El = dpool.tile([128, 16], f32, name="El")
    TT(out=Eh[:], in0=s_[:, 16:32], in1=pwf[:, 16:32], op=OP.mult)
    TT(out=El[:], in0=el[:, 16:32], in1=pwf[:, 16:32], op=OP.mult)
    # Knuth two-sum: (Ah, Al) = 1 + (Eh, El)
    def dt16(name):
        return dpool.tile([128, 16], f32, name=name)
    Ah, Al, z_, t1_, t2_, t3_ = dt16("Ah"), dt16("Al"), dt16("z_"), dt16("t1_"), dt16("t2_"), dt16("t3_")
    TS(out=Ah[:], in0=Eh[:], scalar1=1.0, scalar2=None, op0=OP.add)
    TS(out=z_[:], in0=Ah[:], scalar1=1.0, scalar2=None, op0=OP.subtract)
    TT(out=t1_[:], in0=Ah[:], in1=z_[:], op=OP.subtract)
    TS(out=t2_[:], in0=t1_[:], scalar1=-1.0, scalar2=1.0, op0=OP.mult, op1=OP.add)
    TT(out=t3_[:], in0=Eh[:], in1=z_[:], op=OP.subtract)
    TT(out=t2_[:], in0=t2_[:], in1=t3_[:], op=OP.add)
    TT(out=Al[:], in0=t2_[:], in1=El[:], op=OP.add)
    TT(out=t1_[:], in0=Ah[:], in1=Al[:], op=OP.add)
    TT(out=t2_[:], in0=t1_[:], in1=Ah[:], op=OP.subtract)
    TT(out=Al[:], in0=Al[:], in1=t2_[:], op=OP.subtract)
    CP(out=Ah[:], in_=t1_[:])

    # Dekker product D = (Ac) x (Ak) -> pack3 (Dhi | Dlo | flat)
    pack3 = dpool.tile([128, 24], f32, name="pack3")
    Dhi, Dlo, flatf = pack3[:, 0:8], pack3[:, 8:16], pack3[:, 16:24]
    ach, akh = Ah[:, 0:8], Ah[:, 8:16]
    acl, akl = Al[:, 0:8], Al[:, 8:16]
    t0, t1, er = dt("t0"), dt("t1"), dt("er")
    h1, l1, h2, l2 = dt("h1"), dt("l1"), dt("h2"), dt("l2")
    TT(out=er[:], in0=ach, in1=akh, op=OP.mult)           # er = Dh for now
    nc.vector.tensor_scalar_mul(out=t0[:], in0=ach, scalar1=4097.0)
    TT(out=t1[:], in0=t0[:], in1=ach, op=OP.subtract)
    TT(out=h1[:], in0=t0[:], in1=t1[:], op=OP.subtract)
    TT(out=l1[:], in0=ach, in1=h1[:], op=OP.subtract)
    nc.vector.tensor_scalar_mul(out=t0[:], in0=akh, scalar1=4097.0)
    TT(out=t1[:], in0=t0[:], in1=akh, op=OP.subtract)
    TT(out=h2[:], in0=t0[:], in1=t1[:], op=OP.subtract)
    TT(out=l2[:], in0=akh, in1=h2[:], op=OP.subtract)
    Dh = dt("Dh")
    CP(out=Dh[:], in_=er[:])
    TT(out=er[:], in0=h1[:], in1=h2[:], op=OP.mult)
    TT(out=er[:], in0=er[:], in1=Dh[:], op=OP.subtract)
    TT(out=t0[:], in0=h1[:], in1=l2[:], op=OP.mult)
    TT(out=er[:], in0=er[:], in1=t0[:], op=OP.add)
    TT(out=t0[:], in0=l1[:], in1=h2[:], op=OP.mult)
    TT(out=er[:], in0=er[:], in1=t0[:], op=OP.add)
    TT(out=t0[:], in0=l1[:], in1=l2[:], op=OP.mult)
    TT(out=er[:], in0=er[:], in1=t0[:], op=OP.add)
    TT(out=t0[:], in0=ach, in1=akl, op=OP.mult)
    TT(out=t1[:], in0=akh, in1=acl, op=OP.mult)
    TT(out=t0[:], in0=t0[:], in1=t1[:], op=OP.add)
    TT(out=er[:], in0=er[:], in1=t0[:], op=OP.add)
    # normalize (Dh + er) -> (Dhi, Dlo)
    TT(out=t0[:], in0=Dh[:], in1=er[:], op=OP.add)
    TT(out=t1[:], in0=t0[:], in1=Dh[:], op=OP.subtract)
    TT(out=Dlo, in0=er[:], in1=t1[:], op=OP.subtract)
    CP(out=Dhi, in_=t0[:])
    CP(out=flatf, in_=cidx[:])                            # u32 -> f32 convert

    # ---------------- decode boxes (reference arithmetic order) -------------
    sx, sy = dt("sx"), dt("sy")
    nc.scalar.activation(out=sx[:], in_=raw3[:, :, 0], func=SIG)
    nc.scalar.activation(out=sy[:], in_=raw3[:, :, 1], func=SIG)
    xc, yc, wv, hv, hw, hh = dt("xc"), dt("yc"), dt("wv"), dt("hv"), dt("hw"), dt("hh")
    TT(out=xc[:], in0=sx[:], in1=gxf[:], op=OP.add)
    TT(out=xc[:], in0=xc[:], in1=stf[:], op=OP.mult)
    TT(out=yc[:], in0=sy[:], in1=gyf[:], op=OP.add)
    TT(out=yc[:], in0=yc[:], in1=stf[:], op=OP.mult)
    TT(out=wv[:], in0=ew[:, 0:8], in1=awf[:], op=OP.mult)
    TT(out=hv[:], in0=ew[:, 8:16], in1=ahf[:], op=OP.mult)
    nc.vector.tensor_scalar_mul(out=hw[:], in0=wv[:], scalar1=0.5)
    nc.vector.tensor_scalar_mul(out=hh[:], in0=hv[:], scalar1=0.5)

    # rows6 fields: x1, y1, x2, y2, score, area   (block-major, 6 per block)
    rows6 = dpool.tile([128, 48], f32, name="rows6")
    r63 = rows6[:].rearrange("p (b f) -> p b f", f=6)
    TT(out=r63[:, :, 0], in0=xc[:], in1=hw[:], op=OP.subtract)
    TT(out=r63[:, :, 1], in0=yc[:], in1=hh[:], op=OP.subtract)
    TT(out=r63[:, :, 2], in0=xc[:], in1=hw[:], op=OP.add)
    TT(out=r63[:, :, 3], in0=yc[:], in1=hh[:], op=OP.add)
    # output score: ACT sigmoid product (|err| ~1e-5, gate is 2e-2)
    sc_, sk_ = dt("sc_"), dt("sk_")
    nc.scalar.activation(out=sc_[:], in_=raw3[:, :, 4], func=SIG)
    nc.scalar.activation(out=sk_[:], in_=raw3[:, :, 5], func=SIG)
    TT(out=r63[:, :, 4], in0=sc_[:], in1=sk_[:], op=OP.mult)
    dx, dy = dt("dx"), dt("dy")
    TT(out=dx[:], in0=r63[:, :, 2], in1=r63[:, :, 0], op=OP.subtract)
    nc.vector.tensor_scalar_max(out=dx[:], in0=dx[:], scalar1=0.0)
    TT(out=dy[:], in0=r63[:, :, 3], in1=r63[:, :, 1], op=OP.subtract)
    nc.vector.tensor_scalar_max(out=dy[:], in0=dy[:], scalar1=0.0)
    TT(out=r63[:, :, 5], in0=dx[:], in1=dy[:], op=OP.mult)

    if DEBUG_DUMP:
        dbg = nc.dram_tensor("dbg", [128 * 296], f32, kind="ExternalOutput")

        def dump(off, ap_, w):
            nc.sync.dma_start(
                out=dbg.ap()[128 * off : 128 * (off + w)].rearrange(
                    "(p c) -> p c", c=w
                ),
                in_=ap_,
            )

        dump(0, cidx[:].bitcast(f32), 8)        # u32 bits
        dump(8, rawidx[:].bitcast(f32), 8)      # u32 bits
        dump(224, S[:, 0:16], 16)
        dump(240, tk[:, 0:32].bitcast(f32), 32)
        dump(272, slotidx[:].bitcast(f32), 8)
        dump(280, S[:, 2000:2016], 16)
        dump(16, raw[:], 48)
        dump(64, pack3[:], 24)
        dump(88, rows6[:], 48)
        dump(136, gxf[:], 8)
        dump(144, gyf[:], 8)
        dump(152, awf[:], 8)
        dump(160, ahf[:], 8)
        dump(168, stf[:], 8)
        dump(176, ew[:], 16)
        dump(192, Ah[:], 16)
        dump(208, Al[:], 16)

    # ---------------- stage F: per-image rank / sort / NMS ----------------
    tpool = es.enter_context(tc.tile_pool(name="tpool", bufs=1))
    tq = es.enter_context(tc.tile_pool(name="tq", bufs=1, space="PSUM"))
    T_ps = tq.tile([24, 128], f32, name="T_ps")
    nc.tensor.matmul(out=T_ps[:], lhsT=pack3[:], rhs=ident_sb[:], start=True, stop=True)
    # T6 = transposed keys, replicated 4x along free axis (for the gate trick)
    T6 = tpool.tile([24, 512], f32, name="T6")
    for r_ in range(4):
        nc.scalar.copy(out=T6[:, 128 * r_ : 128 * (r_ + 1)], in_=T_ps[:])
    T_rep6 = T6[:]

    mpool = es.enter_context(tc.tile_pool(name="mpool", bufs=2))
    qpool = es.enter_context(tc.tile_pool(name="qpool", bufs=1, space="PSUM"))
    qone = qpool
    for i in range(IPC):
        # jmat [128, 768] = broadcast of this image's (Dhi|Dlo|flat) keys:
        # gate the transposed key rows with the image mask, then one
        # all-ones matmul sums the single live row per column block.
        gj = mpool.tile([24, 512], f32, tag="gj", name=f"gj_{i}")
        TT(out=gj[:], in0=T_rep6, in1=maskJ_sb[:, 512 * i : 512 * (i + 1)], op=OP.mult)
        jmat = qpool.tile([128, 512], f32, tag="jmat", name=f"jmat_{i}")
        nc.tensor.matmul(
            out=jmat[:], lhsT=ones24_sb[:], rhs=gj[:], start=True, stop=True
        )
        jhi = jmat[:, 0:256]
        jlo = jmat[:, 256:512]
        # rank: cnt = #{j : D_j < D_i} (ascending D = descending score);
        # exact (Dhi, Dlo) double-ties are ~2^-48 and skipped.
        rank = mpool.tile([128, 2], f32, tag="rank", name=f"rank_{i}")
        for c_ in range(2):
            col = 2 * i + c_
            a1 = mpool.tile([128, 256], f32, tag="a1", name=f"a1_{i}{c_}")
            a2 = mpool.tile([128, 256], f32, tag="a2", name=f"a2_{i}{c_}")
            a3 = mpool.tile([128, 256], f32, tag="a3", name=f"a3_{i}{c_}")
            TS(out=a1[:], in0=jhi, scalar1=pack3[:, col : col + 1],
               scalar2=None, op0=OP.is_lt)
            TS(out=a2[:], in0=jhi, scalar1=pack3[:, col : col + 1],
               scalar2=None, op0=OP.is_equal)
            TS(out=a3[:], in0=jlo, scalar1=pack3[:, 8 + col : 8 + col + 1],
               scalar2=None, op0=OP.is_lt)
            TT(out=a2[:], in0=a2[:], in1=a3[:], op=OP.mult)
            TT(out=a1[:], in0=a1[:], in1=a2[:], op=OP.add)
            nc.vector.reduce_sum(
                out=rank[:, c_ : c_ + 1], in_=a1[:], axis=mybir.AxisListType.X
            )
        # one-hot P[cand, r] = (rank_cand == r), r in [0,128)
        s6p = qpool.tile([128, 6], f32, tag="s6p", name=f"s6p_{i}")
        for c_ in range(2):
            P = mpool.tile([128, 128], f32, tag="P", name=f"P_{i}{c_}")
            TS(out=P[:], in0=iota_sb[:], scalar1=rank[:, c_ : c_ + 1],
               scalar2=None, op0=OP.is_equal)
            nc.tensor.matmul(
                out=s6p[:],
                lhsT=P[:],
                rhs=rows6[:, 12 * i + 6 * c_ : 12 * i + 6 * c_ + 6],
                start=(c_ == 0), stop=(c_ == 1),
            )
        s6 = mpool.tile([128, 6], f32, tag="s6", name=f"s6_{i}")
        nc.scalar.copy(out=s6[:], in_=s6p[:])

        # replicate sorted columns via PE: transpose s6, gate with the field
        # mask, one all-ones matmul -> jbA = x1 | y1 | x2 | y2, jbB = area.
        # jbA reuses the jmat PSUM slot (jmat is dead after the rank ops).
        jbA = qpool.tile([128, 512], f32, tag="jbA", name=f"jbA_{i}")
        jbB = qpool.tile([128, 128], f32, tag="jbB", name=f"jbB_{i}")
        nc.tensor.matmul(
            out=jbA[0:6, 0:128], lhsT=s6[:], rhs=ident_sb[:], start=True, stop=True
        )
        g2 = mpool.tile([6, 640], f32, tag="g2", name=f"g2_{i}")
        for r_ in range(5):
            nc.scalar.copy(out=g2[:, 128 * r_ : 128 * (r_ + 1)], in_=jbA[0:6, 0:128])
        TT(out=g2[:], in0=g2[:], in1=maskB_sb[0:6, :], op=OP.mult)
        nc.tensor.matmul(
            out=jbA[:], lhsT=ones24_sb[0:6, :], rhs=g2[:, 0:512],
            start=True, stop=True,
        )
        nc.tensor.matmul(
            out=jbB[:], lhsT=ones24_sb[0:6, :], rhs=g2[:, 512:640],
            start=True, stop=True,
        )
        # IoU suppression matrix, i = partition (suppressor rank), j = free
        ltx = mpool.tile([128, 128], f32, tag="ltx", name=f"ltx_{i}")
        lty = mpool.tile([128, 128], f32, tag="lty", name=f"lty_{i}")
        rbx = mpool.tile([128, 128], f32, tag="rbx", name=f"rbx_{i}")
        rby = mpool.tile([128, 128], f32, tag="rby", name=f"rby_{i}")
        TS(out=ltx[:], in0=jbA[:, 0:128], scalar1=s6[:, 0:1], scalar2=None, op0=OP.max)
        TS(out=lty[:], in0=jbA[:, 128:256], scalar1=s6[:, 1:2], scalar2=None, op0=OP.max)
        TS(out=rbx[:], in0=jbA[:, 256:384], scalar1=s6[:, 2:3], scalar2=None, op0=OP.min)
        TS(out=rby[:], in0=jbA[:, 384:512], scalar1=s6[:, 3:4], scalar2=None, op0=OP.min)
        TT(out=ltx[:], in0=rbx[:], in1=ltx[:], op=OP.subtract)
        nc.vector.tensor_scalar_max(out=ltx[:], in0=ltx[:], scalar1=0.0)
        TT(out=lty[:], in0=rby[:], in1=lty[:], op=OP.subtract)
        nc.vector.tensor_scalar_max(out=lty[:], in0=lty[:], scalar1=0.0)
        inter = mpool.tile([128, 128], f32, tag="inter", name=f"inter_{i}")
        TT(out=inter[:], in0=ltx[:], in1=lty[:], op=OP.mult)
        un = mpool.tile([128, 128], f32, tag="un", name=f"un_{i}")
        TS(out=un[:], in0=jbB[:], scalar1=s6[:, 5:6], scalar2=None, op0=OP.add)
        TT(out=un[:], in0=un[:], in1=inter[:], op=OP.subtract)
        TS(out=un[:], in0=un[:], scalar1=1e-9, scalar2=0.5, op0=OP.add, op1=OP.mult)
        M = mpool.tile([128, 128], f32, tag="M", name=f"M_{i}")
        TT(out=M[:], in0=inter[:], in1=un[:], op=OP.is_gt)
        # lower-triangular mask: keep only i < j (earlier rank suppresses later)
        nc.gpsimd.affine_select(
            out=M[:], in_=M[:], pattern=[[1, 128]], base=0,
            channel_multiplier=-1, compare_op=OP.is_gt, fill=0.0,
        )
        # fixed-point greedy-NMS keep flags
        Kv = mpool.tile([128, 1], f32, tag="Kv", name=f"Kv_{i}")
        nc.vector.memset(Kv[:], 1.0)
        for it in range(NMS_ITERS):
            sup = qone.tile([128, 1], f32, tag="sp1", name=f"sup_{i}_{it}")
            nc.tensor.matmul(out=sup[:], lhsT=M[:], rhs=Kv[:], start=True, stop=True)
            TS(out=Kv[:], in0=sup[:], scalar1=0.0, scalar2=None, op0=OP.is_equal)
        # compact first 100 kept rows to the output
        ps = qone.tile([128, 1], f32, tag="sp1", name=f"ps_{i}")
        nc.tensor.matmul(out=ps[:], lhsT=ltri_sb[:], rhs=Kv[:], start=True, stop=True)
        psm1 = mpool.tile([128, 1], f32, tag="psm1", name=f"psm1_{i}")
        nc.vector.tensor_scalar_sub(out=psm1[:], in0=ps[:], scalar1=1.0)
        O = mpool.tile([128, 128], f32, tag="O", name=f"O_{i}")
        TS(out=O[:], in0=iota_sb[:], scalar1=psm1[:], scalar2=None, op0=OP.is_equal)
        TT(out=O[:], in0=O[:], in1=Kv[:].to_broadcast([128, 128]), op=OP.mult)
        outp = qpool.tile([128, 6], f32, tag="s6p", name=f"outp_{i}")
        nc.tensor.matmul(
            out=outp[0:MAXP, 0:5], lhsT=O[:, 0:MAXP], rhs=s6[:, 0:5],
            start=True, stop=True,
        )
        osb = mpool.tile([MAXP, 5], f32, tag="osb", name=f"osb_{i}")
        nc.scalar.copy(out=osb[:], in_=outp[0:MAXP, 0:5])
        nc.sync.dma_start(
            out=out_ap[i * MAXP * 5 : (i + 1) * MAXP * 5].rearrange(
                "(p f) -> p f", f=5
            ),
            in_=osb[:],
        )


@functools.cache
def build_nc() -> bass.Bass:
    nc = bacc.Bacc(
        "TRN2", target_bir_lowering=False, debug=False,
        enable_asserts=False, num_devices=CORES,
    )
    x = nc.dram_tensor("x", [IPC * N * 6], f32, kind="ExternalInput")
    s = nc.dram_tensor("s", [2 * 128 * F], f32, kind="ExternalInput")
    out = nc.dram_tensor("out", [IPC * MAXP * 5], f32, kind="ExternalOutput")
    st = nc.dram_tensor("st", [2048], u32, kind="Internal")
    with tile.TileContext(nc) as tc:
        with ExitStack() as es:
            _body(nc, tc, es, x, s, out, st)
    nc.compile()
    return nc


@functools.cache
def _host_idx() -> np.ndarray:
    """[2, 128, F] int64 gather indices into the per-core flat input (+sentinel)."""
    p_img = np.arange(32)[:, None]
    c = np.arange(F)[None, :]
    g = p_img * F + c                       # [32, F]
    real = g < N
    idx = np.zeros((2, 128, F), np.int64)
    for f in range(2):
        for i in range(IPC):
            base = i * N * 6
            gi = np.where(real, base + g * 6 + 4 + f, IPC * N * 6)
            idx[f, 32 * i : 32 * i + 32, :] = gi
    return idx


def _host_prep(p2, p3, p4, p5) -> list[dict[str, np.ndarray]]:
    flat = np.concatenate(
        [p.reshape(B, -1, 6) for p in (p2, p3, p4, p5)], axis=1
    ).astype(np.float32, copy=False)  # [B, N, 6]
    idx = _host_idx()
    in_maps = []
    for c in range(CORES):
        xc = np.ascontiguousarray(flat[c * IPC : (c + 1) * IPC]).reshape(-1)
        xx = np.concatenate([xc, np.array([FILL], np.float32)])
        sc = np.ascontiguousarray(xx[idx])
        in_maps.append({"x": xc, "s": sc.reshape(-1)})
    return in_maps


def kernel(p2, p3, p4, p5) -> np.ndarray:
    nc = build_nc()
    in_maps = _host_prep(p2, p3, p4, p5)
    res = run_bass_kernel_spmd(nc, in_maps, core_ids=list(range(CORES)))
    outs = [r["out"].reshape(IPC, MAXP, 5) for r in res.results]
    return np.concatenate(outs, axis=0).astype(np.float32)
